# revision 8
# baseline (speedup 1.0000x reference)
"""BertCrfForTokenClassification loss kernel for 8 TRN2 NeuronCores.

Data-parallel over batch: 8 sequences per core. Per core:
  - emissions em = hid @ W + b via PE (transpose hid tiles, then matmul)
  - CRF denominator via blocked exp-space scan on DVE:
      partitions p = g*8 + b  (g = time-block of 32 steps, b = local seq)
      phase 1: per-partition product of 32 per-step 9x9 matrices
               M_t = exp(trans) * exp(em_t) (masked steps -> identity),
               rescaled by max each round (log-scales accumulated)
      phase 2: sequential combine of the 16 block products per sequence
  - CRF numerator (gold path score) via one-hot algebra, fully vectorized
  - output: per-core scalar  sum_b (denominator_b - numerator_b)
Host sums the 8 scalars and divides by 64.
"""
import sys, os
sys.path.insert(0, "/opt/trn_rl_repo")
import numpy as np
import ml_dtypes

B, T, H, K = 64, 512, 768, 9
NB, G, TB = 8, 16, 32          # seqs/core, blocks, block length
NGRP, GT = 8, 4                # t-groups of 4 local steps
F32 = np.float32
BF16 = ml_dtypes.bfloat16

_prog = None


def _build():
    from concourse import bacc, mybir, tile
    import concourse.bass as bass

    f32 = mybir.dt.float32
    bf16 = mybir.dt.bfloat16
    AFT = mybir.ActivationFunctionType
    ALU = mybir.AluOpType
    AX = mybir.AxisListType
    EXP = AFT.Exp
    LN = getattr(AFT, "Ln", None) or getattr(AFT, "Log")

    nc = bacc.Bacc("TRN2", target_bir_lowering=False, debug=False,
                   enable_asserts=False, num_devices=8)

    hid_d = nc.dram_tensor("hid", [128, TB * H], f32, kind="ExternalInput").ap()
    msk_d = nc.dram_tensor("msk", [128, TB], f32, kind="ExternalInput").ap()
    lab_d = nc.dram_tensor("lab", [128, TB], f32, kind="ExternalInput").ap()
    wc_d = nc.dram_tensor("wc", [128, 54], f32, kind="ExternalInput").ap()
    cf_d = nc.dram_tensor("cf", [128, 36], f32, kind="ExternalInput").ap()
    cb_d = nc.dram_tensor("cb", [128, 252], bf16, kind="ExternalInput").ap()
    idm_d = nc.dram_tensor("idm", [128, 128], f32, kind="ExternalInput").ap()
    out_d = nc.dram_tensor("out", [1, 1], f32, kind="ExternalOutput").ap()

    with tile.TileContext(nc) as tc:
        with (
            tc.tile_pool(name="const", bufs=1) as cpool,
            tc.tile_pool(name="pers", bufs=1) as pp,
            tc.tile_pool(name="hin", bufs=3) as ph,
            tc.tile_pool(name="ht", bufs=3) as pht,
            tc.tile_pool(name="emt", bufs=2) as pemt,
            tc.tile_pool(name="ptr", bufs=2, space="PSUM") as pptr,
            tc.tile_pool(name="pem", bufs=2, space="PSUM") as ppem,
            tc.tile_pool(name="pbt", bufs=2, space="PSUM") as ppbt,
            tc.tile_pool(name="pfin", bufs=1, space="PSUM") as ppfin,
        ):
            w_sb = cpool.tile([128, 54], f32)
            cf_sb = cpool.tile([128, 36], f32)
            cb_sb = cpool.tile([128, 252], bf16)
            id_sb = cpool.tile([128, 128], f32)
            nc.sync.dma_start(w_sb[:], wc_d[:])
            nc.sync.dma_start(cf_sb[:], cf_d[:])
            nc.sync.dma_start(cb_sb[:], cb_d[:])
            nc.sync.dma_start(id_sb[:], idm_d[:])
            KIO, ENO, STO, BKO = 0, 9, 18, 27            # cf slices
            ATO, I9O, TRO, EBO = 0, 81, 162, 243          # cb slices

            m_f = pp.tile([128, TB], f32)
            lab_f = pp.tile([128, TB], f32)
            nc.sync.dma_start(m_f[:], msk_d[:])
            nc.sync.dma_start(lab_f[:], lab_d[:])
            mn = pp.tile([128, TB], f32)
            mz = pp.tile([128, TB], mybir.dt.int32)
            nc.vector.tensor_copy(mn[:], m_f[:])
            nc.vector.memset(mn[0:8, 0:1], 0.0)           # global t=0 handled by alpha0
            nc.vector.tensor_scalar(mz[:], mn[:], -1.0, 1.0,
                                    op0=ALU.mult, op1=ALU.add)

            em_scan = pp.tile([128, TB * K], f32)
            e_bf = pp.tile([128, TB * K], bf16)
            Mfull = pp.tile([128, TB * 81], bf16)
            Pbf = pp.tile([128, 81], bf16)
            P32 = pp.tile([128, 81], f32)
            S = pp.tile([128, 729], bf16)
            mx = pp.tile([128, 1], f32)
            rc = pp.tile([128, 1], f32)
            lg = pp.tile([128, 1], f32)
            ls = pp.tile([128, 1], f32)
            nc.vector.memset(ls[:], 0.0)

            Mv_all = Mfull.rearrange("p (t j l) -> p t j l", j=9, l=9)
            ev_all = e_bf.rearrange("p (t j) -> p t j", j=9)
            Av = cb_sb[:, ATO:ATO + 81].rearrange("p (j l) -> p j l", l=9)
            Iv = cb_sb[:, I9O:I9O + 81].rearrange("p (j l) -> p j l", l=9)

            for grp in range(NGRP):
                t0 = grp * GT
                hid_s = ph.tile([128, GT * H], f32)
                nc.sync.dma_start(hid_s[:], hid_d[:, t0 * H:(t0 + GT) * H])
                hidT = pht.tile([128, 6 * 512], f32)
                for c in range(6):
                    ps = pptr.tile([128, 512], f32)
                    for tt in range(GT):
                        nc.tensor.transpose(
                            ps[:, tt * 128:(tt + 1) * 128],
                            hid_s[:, tt * H + c * 128: tt * H + c * 128 + 128],
                            id_sb[:])
                    dst = hidT[:, c * 512:(c + 1) * 512]
                    if c % 2 == 0:
                        nc.vector.tensor_copy(dst, ps[:])
                    else:
                        nc.scalar.copy(dst, ps[:])
                pe = ppem.tile([9, 512], f32)
                for c in range(6):
                    nc.tensor.matmul(pe[:], w_sb[:, c * 9:(c + 1) * 9],
                                     hidT[:, c * 512:(c + 1) * 512],
                                     start=(c == 0), stop=(c == 5))
                emT = pemt.tile([9, 512], f32)
                nc.scalar.copy(emT[:], pe[:])
                for tt in range(GT):
                    t = t0 + tt
                    pb = ppbt.tile([128, 9], f32)
                    nc.tensor.transpose(pb[:], emT[:, tt * 128:tt * 128 + 128],
                                        id_sb[0:9, 0:9])
                    nc.vector.tensor_add(em_scan[:, t * 9:(t + 1) * 9], pb[:],
                                         cf_sb[:, BKO:BKO + 9])
                nc.scalar.activation(e_bf[:, t0 * 9:(t0 + GT) * 9],
                                     em_scan[:, t0 * 9:(t0 + GT) * 9], EXP)
                Mv = Mv_all[:, t0:t0 + GT]
                nc.vector.tensor_mul(
                    Mv,
                    Av[:, None, :, :].broadcast_to([128, GT, 9, 9]),
                    ev_all[:, t0:t0 + GT][:, :, :, None].broadcast_to([128, GT, 9, 9]))
                nc.vector.copy_predicated(
                    Mv,
                    mz[:, t0:t0 + GT][:, :, None, None].broadcast_to([128, GT, 9, 9]),
                    Iv[:, None, :, :].broadcast_to([128, GT, 9, 9]))
                # scan rounds for this group
                for tt in range(GT):
                    t = t0 + tt
                    if t == 0:
                        nc.vector.tensor_copy(Pbf[:], Mfull[:, 0:81])
                        continue
                    Sv = S.rearrange("p (i j l) -> p i j l", j=9, l=9)
                    nc.vector.tensor_mul(
                        Sv,
                        Pbf.rearrange("p (i l) -> p i l", l=9)
                           [:, :, None, :].broadcast_to([128, 9, 9, 9]),
                        Mv_all[:, t][:, None, :, :].broadcast_to([128, 9, 9, 9]))
                    nc.vector.tensor_reduce(
                        P32[:], S.rearrange("p (x l) -> p x l", l=9),
                        axis=AX.X, op=ALU.add)
                    if t % 2 == 0 or t == TB - 1:
                        nc.vector.tensor_reduce(mx[:], P32[:], axis=AX.X, op=ALU.max)
                        nc.vector.reciprocal(rc[:], mx[:])
                        nc.vector.tensor_scalar_mul(Pbf[:], P32[:], rc[:])
                        nc.scalar.activation(lg[:], mx[:], LN)
                        nc.vector.tensor_add(ls[:], ls[:], lg[:])
                    else:
                        nc.vector.tensor_copy(Pbf[:], P32[:])

            # ---------------- phase 2 ----------------
            al = pp.tile([128, 9], bf16)
            al32 = pp.tile([128, 9], f32)
            tmp9 = pp.tile([128, 9], f32)
            S2 = pp.tile([128, 81], bf16)
            ls2 = pp.tile([128, 1], f32)
            Pseq = pp.tile([8, G * 81], bf16)
            for g in range(G):
                nc.sync.dma_start(Pseq[0:8, g * 81:(g + 1) * 81],
                                  Pbf[g * 8:(g + 1) * 8, :])
            nc.vector.memset(ls2[:], 0.0)
            nc.vector.tensor_add(tmp9[0:8, :], em_scan[0:8, 0:9],
                                 cf_sb[0:8, STO:STO + 9])
            nc.scalar.activation(al[0:8, :], tmp9[0:8, :], EXP)
            for g in range(G):
                S2v = S2.rearrange("p (j i) -> p j i", i=9)[0:8]
                nc.vector.tensor_mul(
                    S2v,
                    al[0:8][:, None, :].broadcast_to([8, 9, 9]),
                    Pseq[0:8, g * 81:(g + 1) * 81]
                        .rearrange("p (i j) -> p i j", j=9)
                        .rearrange("p i j -> p j i"))
                nc.vector.tensor_reduce(
                    al32[0:8, :], S2.rearrange("p (x i) -> p x i", i=9)[0:8],
                    axis=AX.X, op=ALU.add)
                nc.vector.tensor_reduce(mx[0:8, :], al32[0:8, :],
                                        axis=AX.X, op=ALU.max)
                nc.vector.reciprocal(rc[0:8, :], mx[0:8, :])
                nc.vector.tensor_scalar_mul(al[0:8, :], al32[0:8, :], rc[0:8, :])
                nc.scalar.activation(lg[0:8, :], mx[0:8, :], LN)
                nc.vector.tensor_add(ls2[0:8, :], ls2[0:8, :], lg[0:8, :])
            nc.vector.tensor_mul(tmp9[0:8, :], al[0:8, :],
                                 cb_sb[0:8, EBO:EBO + 9])
            nc.vector.tensor_reduce(mx[0:8, :], tmp9[0:8, :],
                                    axis=AX.X, op=ALU.add)
            nc.scalar.activation(lg[0:8, :], mx[0:8, :], LN)
            nc.vector.tensor_add(ls2[0:8, :], ls2[0:8, :], lg[0:8, :])

            # ---------------- numerator ----------------
            OHf = pp.tile([128, TB * K], f32)
            OHb = pp.tile([128, TB * K], bf16)
            OHp = pp.tile([128, TB * K], bf16)
            S3 = pp.tile([128, TB * 81], bf16)
            R = pp.tile([128, TB * K], f32)
            X = pp.tile([128, TB * K], f32)
            Y = pp.tile([128, TB * K], f32)
            t32a = pp.tile([128, TB], f32)
            t32b = pp.tile([128, TB], f32)
            mnx = pp.tile([128, TB], f32)
            n1 = pp.tile([128, 1], f32)
            n2 = pp.tile([128, 1], f32)
            sgt = pp.tile([128, 1], f32)
            vv = pp.tile([128, 1], f32)
            ww = pp.tile([128, 1], f32)
            ones = pp.tile([128, 1], f32)

            nc.vector.tensor_tensor(
                OHf.rearrange("p (t k) -> p t k", k=9),
                lab_f[:, :, None].broadcast_to([128, TB, 9]),
                cf_sb[:, KIO:KIO + 9][:, None, :].broadcast_to([128, TB, 9]),
                op=ALU.is_equal)
            nc.vector.tensor_copy(OHb[:], OHf[:])
            nc.vector.tensor_copy(OHp[:, 9:TB * K], OHb[:, 0:(TB - 1) * K])
            nc.sync.dma_start(OHp[8:128, 0:9], OHb[0:120, (TB - 1) * K:TB * K])
            nc.vector.memset(OHp[0:8, 0:9], 0.0)
            nc.vector.tensor_mul(
                S3.rearrange("p (t j i) -> p t j i", j=9, i=9),
                OHp.rearrange("p (t i) -> p t i", i=9)
                   [:, :, None, :].broadcast_to([128, TB, 9, 9]),
                cb_sb[:, TRO:TRO + 81].rearrange("p (i j) -> p i j", j=9)
                     .rearrange("p i j -> p j i")
                     [:, None, :, :].broadcast_to([128, TB, 9, 9]))
            nc.vector.tensor_reduce(R[:], S3.rearrange("p (x i) -> p x i", i=9),
                                    axis=AX.X, op=ALU.add)
            nc.vector.tensor_add(X[:], R[:], em_scan[:])
            nc.vector.tensor_mul(Y[:], X[:], OHf[:])
            nc.vector.tensor_reduce(t32a[:], Y.rearrange("p (t k) -> p t k", k=9),
                                    axis=AX.X, op=ALU.add)
            nc.vector.tensor_mul(t32b[:], t32a[:], mn[:])
            nc.vector.tensor_reduce(n1[:], t32b[:], axis=AX.X, op=ALU.add)
            nc.vector.tensor_mul(
                Y.rearrange("p (t k) -> p t k", k=9),
                OHf.rearrange("p (t k) -> p t k", k=9),
                cf_sb[:, ENO:ENO + 9][:, None, :].broadcast_to([128, TB, 9]))
            nc.vector.tensor_reduce(t32a[:], Y.rearrange("p (t k) -> p t k", k=9),
                                    axis=AX.X, op=ALU.add)
            nc.vector.tensor_copy(mnx[:, 0:TB - 1], m_f[:, 1:TB])
            nc.vector.memset(mnx[:, TB - 1:TB], 0.0)
            nc.sync.dma_start(mnx[0:120, TB - 1:TB], m_f[8:128, 0:1])
            nc.vector.tensor_sub(t32b[:], m_f[:], mnx[:])
            nc.vector.tensor_mul(t32b[:], t32a[:], t32b[:])
            nc.vector.tensor_reduce(n2[:], t32b[:], axis=AX.X, op=ALU.add)
            nc.vector.tensor_add(tmp9[0:8, :], em_scan[0:8, 0:9],
                                 cf_sb[0:8, STO:STO + 9])
            nc.vector.tensor_mul(tmp9[0:8, :], tmp9[0:8, :], OHf[0:8, 0:9])
            nc.vector.tensor_reduce(sgt[0:8, :], tmp9[0:8, :],
                                    axis=AX.X, op=ALU.add)

            # ---------------- combine & output ----------------
            nc.vector.tensor_sub(vv[:], ls[:], n1[:])
            nc.vector.tensor_sub(vv[:], vv[:], n2[:])
            nc.vector.tensor_sub(ww[0:8, :], ls2[0:8, :], sgt[0:8, :])
            nc.vector.tensor_add(vv[0:8, :], vv[0:8, :], ww[0:8, :])
            nc.vector.memset(ones[:], 1.0)
            pf = ppfin.tile([1, 1], f32)
            nc.tensor.matmul(pf[:], vv[:], ones[:], start=True, stop=True)
            res = pp.tile([1, 1], f32)
            nc.scalar.copy(res[:], pf[:])
            nc.sync.dma_start(out_d[:], res[:])

    nc.compile()
    return nc


def _prep_maps(hidden, attention_mask, labels, W, b, transitions,
               start_transitions, end_transitions):
    hidden = np.ascontiguousarray(hidden, dtype=F32)
    W = np.asarray(W, F32)
    b = np.asarray(b, F32)
    tr = np.asarray(transitions, F32)
    st = np.asarray(start_transitions, F32)
    en = np.asarray(end_transitions, F32)

    wc = np.ascontiguousarray(W.reshape(6, 128, 9).transpose(1, 0, 2)
                              .reshape(128, 54))
    kio = np.arange(9, dtype=F32)
    cf_row = np.concatenate([kio, en, st, b]).astype(F32)
    cf = np.tile(cf_row[None, :], (128, 1))
    A_T = np.exp(tr).T.reshape(81)
    I9 = np.eye(9, dtype=F32).reshape(81)
    trn = tr.reshape(81)
    Eb = np.exp(en)
    cb_row = np.concatenate([A_T, I9, trn, Eb]).astype(F32)
    cb = np.tile(cb_row[None, :], (128, 1)).astype(BF16)
    idm = np.eye(128, dtype=F32)

    maps = []
    for c in range(8):
        sl = slice(c * NB, (c + 1) * NB)
        hidr = np.ascontiguousarray(
            hidden[sl].reshape(NB, G, TB, H).transpose(1, 0, 2, 3)
            .reshape(128, TB * H))
        mr = np.ascontiguousarray(
            np.asarray(attention_mask[sl], F32).reshape(NB, G, TB)
            .transpose(1, 0, 2).reshape(128, TB))
        lr = np.ascontiguousarray(
            np.asarray(labels[sl], F32).reshape(NB, G, TB)
            .transpose(1, 0, 2).reshape(128, TB))
        maps.append({"hid": hidr, "msk": mr, "lab": lr, "wc": wc,
                     "cf": cf, "cb": cb, "idm": idm})
    return maps


def kernel(hidden, attention_mask, labels, W, b, transitions,
           start_transitions, end_transitions):
    global _prog
    from concourse import bass_utils
    if _prog is None:
        _prog = _build()
    maps = _prep_maps(hidden, attention_mask, labels, W, b, transitions,
                      start_transitions, end_transitions)
    res = bass_utils.run_bass_kernel_spmd(_prog, maps, core_ids=list(range(8)))
    tot = sum(float(r["out"][0, 0]) for r in res.results)
    return np.asarray(tot / B, dtype=F32)


if __name__ == "__main__":
    d = np.load(os.path.join(os.path.dirname(__file__), "inputs.npz"))
    out = kernel(**{k: d[k] for k in d.files})
    print("loss:", out)
